# revision 16
# baseline (speedup 1.0000x reference)
"""Single-head causal attention (B=4, S=4096, E=512, D=64) on 8 trn2 cores.

Sharding: 8 cores = 4 batches x 2 query-interleave groups. Core (b, h)
computes output for batch b, query tiles {h, h+2, ..., h+30} (128 rows
each, 16 tiles = 2048 queries). Each core computes K/V for the full
sequence of its batch from x (duplicated across the batch's core pair --
no cross-core collectives). The host permutes key tiles per core
(pair-swap for h=1) so query tiles sit at even local slots and the
block-causal structure is slot-identical across cores.

Numerics: x and the (16x-scaled) weights are fp8e4; QKV projections run
as fp8 DoubleRow matmuls (2 contraction k-tiles streamed per cycle).
Q^T/K^T are requantized to fp8 with a zero second k-tile plane so the
scores matmuls also run in DoubleRow mode at half cost. The causal mask
inside the diagonal band is applied by accumulating a host-provided
triangular -51200 matrix into the score psum via a bf16 matmul
(replaces per-pair DVE mask multiplies). Softmax exp runs with a
uniform exp(s/8 - 1) shift (cancels in softmax; keeps exp < fp8e4 max):
most pairs on ACT (fp8 output), a subset of full-width pairs on DVE via
a Schraudolph bit-trick (u16 = A*s + B bitcast to bf16 ~= exp). The
attention*V accumulation uses one DoubleRow fp8 matmul per block pair
([V_j0|1; V_j1|1] stationary, exp pair moving) into a [65, 512] psum
(numerator rows 0:64, denominator row 64); Schraudolph pairs use two
bf16 matmuls against a bf16 V copy. Host divides, unscales, adds bv.
"""

import numpy as np
from contextlib import ExitStack

import concourse.mybir as mybir
import concourse.tile as tile
from concourse import bacc
from concourse.bass_utils import run_bass_kernel_spmd
from concourse.masks import make_identity

F32 = mybir.dt.float32
BF16 = mybir.dt.bfloat16
FP8 = mybir.dt.float8e4
U16 = mybir.dt.uint16
AF = mybir.ActivationFunctionType
OP = mybir.AluOpType
DR = mybir.MatmulPerfMode.DoubleRow

B, S, E, D = 4, 4096, 512, 64
P = 128
NT = S // P           # 32 key tiles
KC = S // 512         # 8 key chunks
QC = (S // 2) // 512  # 4 query chunks per core
N_CORES = 8

WSCALE = 16.0         # host scales W (and biases) by 16 for fp8 range
MASKVAL = -51200.0    # -25 * 2048 in score units -> exp ~ e^-26
EXP_SCALE = 1.0 / 2048.0   # score -> exponent (1/8 softmax * 1/256 Wscale)
EXP_BIAS = -1.0            # uniform shift; cancels in softmax
# Schraudolph u16/bf16 exp: u16 = SCH_A * score + SCH_B, bitcast bf16
SCH_A = 128.0 * 1.4426950408889634 * EXP_SCALE
SCH_B = 16256.0 - 128.0 * 1.4426950408889634 - 5.5

# pairs computed on DVE (Schraudolph) per query chunk: full-width
# non-band pairs with small p2 (their key blocks get a bf16 V copy)
DVE_PAIRS = {0: [], 1: [0, 1], 2: [0, 1, 2], 3: [0, 1, 2, 3]}
NBF = 4  # bf16 V copies for pairs 0..NBF-1 (blocks 0..2*NBF-1)

_CACHE: dict = {}


def _build():
    nc = bacc.Bacc(
        "TRN2", target_bir_lowering=False, debug=False, num_devices=N_CORES
    )
    xdr = nc.dram_tensor("xdr", [KC, P, 2, 2, 512], FP8, kind="ExternalInput").ap()
    xq = nc.dram_tensor("xq", [QC, P, 2, 2, 512], FP8, kind="ExternalInput").ap()
    w3 = nc.dram_tensor("w3", [P, 2, 2, 192], FP8, kind="ExternalInput").ap()
    bias2 = nc.dram_tensor("bias2", [P, 2], F32, kind="ExternalInput").ap()
    mtri = nc.dram_tensor("mtri", [P, 2, P], BF16, kind="ExternalInput").ap()
    # rows 0:64 attnT numerator (16x), row 64 softmax denominator
    outT = nc.dram_tensor("outT", [D + 1, S // 2], F32, kind="ExternalOutput").ap()

    with tile.TileContext(nc) as tc, ExitStack() as ctx:
        sb_const = ctx.enter_context(tc.tile_pool(name="const", bufs=1))
        sb_kv = ctx.enter_context(tc.tile_pool(name="kv", bufs=1))
        sb_xk = ctx.enter_context(tc.tile_pool(name="xk", bufs=12))
        sb_exp = ctx.enter_context(tc.tile_pool(name="exp", bufs=5))
        sb_u16 = ctx.enter_context(tc.tile_pool(name="u16", bufs=3))
        ps_misc = ctx.enter_context(tc.tile_pool(name="psm", bufs=2, space="PSUM"))
        ps_sc = ctx.enter_context(tc.tile_pool(name="pssc", bufs=2, space="PSUM"))
        ps_at = ctx.enter_context(tc.tile_pool(name="psat", bufs=2, space="PSUM"))

        # ---------------- constants ----------------
        ebias = sb_const.tile([P, 1], F32)
        nc.gpsimd.memset(ebias[:], float(EXP_BIAS))
        # dummy activation at t~0: absorbs the one-time exp table load
        # while the input DMAs are still in flight
        scratch = sb_const.tile([P, 1], F32)
        nc.scalar.activation(scratch[:], ebias[:], AF.Exp)
        w3t = sb_const.tile([P, 2, 2, 192], FP8)
        nc.sync.dma_start(w3t[:], w3)
        # input DMAs, ordered by first use; all issued up front so the
        # per-chunk projection chains never wait on HBM latency
        xk_tiles = {}
        xq_tiles = {}

        def dma_xk(kc, split=False):
            xk = sb_xk.tile([P, 2, 2, 512], FP8, tag="xk", name=f"xk{kc}")
            if split:
                for i in range(2):
                    nc.sync.dma_start(xk[:, i], xdr[kc, :, i])
            else:
                nc.sync.dma_start(xk[:], xdr[kc])
            xk_tiles[kc] = xk

        def dma_xq(c):
            xqt = sb_xk.tile([P, 2, 2, 512], FP8, tag="xk", name=f"xqt{c}")
            nc.sync.dma_start(xqt[:], xq[c])
            xq_tiles[c] = xqt

        dma_xk(0, split=True)
        b2 = sb_const.tile([P, 2], F32)
        nc.sync.dma_start(b2[:], bias2)
        dma_xq(0)
        mt = sb_const.tile([P, 2, P], BF16)
        nc.sync.dma_start(mt[:], mtri)
        # remaining input loads go through the (otherwise idle) gpsimd
        # DGE queue: cheap dispatch, transfers overlap the SP queue
        for kc, c in ((1, None), (2, None), (None, 1), (3, None),
                      (4, None), (None, 2), (5, None), (6, None),
                      (None, 3), (7, None)):
            if kc is not None:
                xk = sb_xk.tile([P, 2, 2, 512], FP8, tag="xk", name=f"xk{kc}")
                nc.gpsimd.dma_start(xk[:], xdr[kc])
                xk_tiles[kc] = xk
            else:
                xqt = sb_xk.tile([P, 2, 2, 512], FP8, tag="xk", name=f"xqt{c}")
                nc.gpsimd.dma_start(xqt[:], xq[c])
                xq_tiles[c] = xqt
        identF = sb_const.tile([P, P], F32)
        make_identity(nc, identF[:])
        # [64,64] fp8 identity at partition base 64 (transpose rhs must
        # share the stationary operand's base partition)
        ident8 = sb_const.tile([P, D], FP8)
        nc.vector.tensor_copy(ident8[D:P, :], identF[0:D, 0:D])
        identM = sb_const.tile([P, P], BF16)
        nc.vector.tensor_copy(identM[:], identF[:])
        bconst = sb_const.tile([P, 1], F32)
        nc.gpsimd.memset(bconst[:], float(SCH_B))

        # ---------------- persistent state ----------------
        # K^T (fp8, 16x, +16bk) on partitions 0:64 / V^T (fp8, 16x) on
        # 64:128; per block j: [p, j, t, col] with t=1 zero (DoubleRow)
        kvdr = sb_kv.tile([P, NT, 2, P], FP8)
        # Q^T fp8 per query chunk: [p(d), c, t, q] with t=1 zero
        qdr = sb_kv.tile([D, QC, 2, 512], FP8)
        # V in [k, d] layout + ones column, paired for DoubleRow AV
        vo = sb_kv.tile([P, NT // 2, 2, D + 1], FP8)
        vobf = sb_kv.tile([P, NBF, 2, D + 1], BF16)
        nc.gpsimd.memset(vo[:, :, :, D], 1.0)
        nc.gpsimd.memset(vobf[:, :, :, D], 1.0)

        def phase_b(kc):
            xk = xk_tiles[kc]
            # zero the DoubleRow t=1 planes for this chunk's K/V blocks
            nc.gpsimd.memset(kvdr[:, 4 * kc : 4 * kc + 4, 1, :], 0.0)
            # [K|V] projection: 4 fp8 DoubleRow matmuls (contraction 128
            # each as 2 k-tiles of 64)
            pkv = ps_misc.tile([P, 512], F32, tag="ps", name=f"pkv{kc}")
            for mi, (hb, i) in enumerate(((0, 0), (0, 1), (1, 0), (1, 1))):
                nc.tensor.matmul(
                    pkv[:],
                    w3t[64 * hb : 64 * hb + 64, i, :, 0:128],
                    xk[64 * hb : 64 * hb + 64, i, :, :],
                    start=(mi == 0),
                    stop=(mi == 3),
                    perf_mode=DR,
                )
            # bias col 0 = 16bk on rows 0:64, zeros on 64:128; fp8 out
            nc.vector.tensor_tensor(
                out=kvdr[:, 4 * kc : 4 * kc + 4, 0, :],
                in0=pkv[:].rearrange("p (b c) -> p b c", c=P),
                in1=b2[:, 0:1].to_broadcast((P, 4, P)),
                op=OP.add,
            )
            return xk

        def phase_b_q(c):
            xqt = xq_tiles[c]
            nc.gpsimd.memset(qdr[:, c, 1, :], 0.0)
            pq = ps_misc.tile([D, 512], F32, tag="ps", name=f"pq{c}")
            for mi, (hb, i) in enumerate(((0, 0), (0, 1), (1, 0), (1, 1))):
                nc.tensor.matmul(
                    pq[:],
                    w3t[64 * hb : 64 * hb + 64, i, :, 128:192],
                    xqt[64 * hb : 64 * hb + 64, i, :, :],
                    start=(mi == 0),
                    stop=(mi == 3),
                    perf_mode=DR,
                )
            nc.vector.tensor_tensor(
                out=qdr[:, c, 0, :],
                in0=pq[:],
                in1=b2[0:D, 1:2].to_broadcast((D, 512)),
                op=OP.add,
            )

        def phase_b_tr(grp):
            # V^T -> V transposes for blocks 8g..8g+7 (fp8: out elem
            # step 2), then one strided DVE copy into vo (+ bf16 copy
            # for the Schraudolph pairs' blocks)
            pt = ps_misc.tile([P, 8, D, 2], FP8, tag="ps", name=f"pt{grp}")
            for bb in range(8):
                j = 8 * grp + bb
                nc.tensor.transpose(
                    pt[:, bb, :, 0],
                    kvdr[D:P, j, 0, :],
                    ident8[D:P, :],
                )
            nc.vector.tensor_copy(
                vo[:, 4 * grp : 4 * grp + 4, :, 0:D],
                pt[:, :, :, 0].rearrange("p (pr two) d -> p pr two d", two=2),
            )
            if 4 * grp < NBF:
                nc.vector.tensor_copy(
                    vobf[:, 4 * grp : 4 * grp + 4, :, 0:D],
                    pt[:, :, :, 0].rearrange("p (pr two) d -> p pr two d", two=2),
                )

        # ---------------- phase C: softmax-attention pipeline ----------
        # Global pair list across all chunks; each pair's scores+exp are
        # emitted 2 slots before its AV so the PE never round-trips
        # through ACT between consecutive exps.
        exp_res = {}
        pat_tiles = {}

        def sc_exp(c, p2):
            j0, j1 = 2 * p2, 2 * p2 + 1
            m = p2 - 4 * c  # band pair index, >= 0 inside the band
            qs = 0 if m < 0 else min(128 * m, 256)  # matmul region
            qe = 0 if m < 0 else 128 * m            # exp/AV region
            psc = ps_sc.tile([P, 1024], F32, tag="sc", name=f"sc{c}_{p2}")
            for ji, j in ((0, j0), (1, j1)):
                nc.tensor.matmul(
                    psc[:, 512 * ji + qs : 512 * ji + 512],
                    kvdr[0:D, j, :, :],
                    qdr[:, c, :, qs:512],
                    start=True,
                    stop=(m < 0),
                    perf_mode=DR,
                )
            if m >= 0:
                # causal band mask: accumulate tri/const -51200 via
                # bf16 matmul into the diagonal 128-col region
                for ji in range(2):
                    nc.tensor.matmul(
                        psc[:, 512 * ji + qe : 512 * ji + qe + P],
                        mt[:, ji, :],
                        identM[:],
                        start=False,
                        stop=True,
                    )
            psc_v = psc[:].rearrange("p (two x) -> p two x", x=512)
            if p2 in DVE_PAIRS[c]:
                u16 = sb_u16.tile([P, 1024], U16, tag="u16", name=f"u{c}_{p2}")
                nc.vector.scalar_tensor_tensor(
                    out=u16[:],
                    in0=psc[:],
                    scalar=float(SCH_A),
                    in1=bconst[:].to_broadcast((P, 1024)),
                    op0=OP.mult,
                    op1=OP.add,
                )
                exp_res[(c, p2)] = ("u16", u16)
            else:
                eT = sb_exp.tile([P, 1024], FP8, tag="eT", name=f"eT{c}_{p2}")
                eT_v = eT[:].rearrange("p (two x) -> p two x", x=512)
                nc.scalar.activation(
                    eT_v[:, :, qe:512],
                    psc_v[:, :, qe:512],
                    AF.Exp,
                    scale=float(EXP_SCALE),
                    bias=ebias[:],
                )
                exp_res[(c, p2)] = ("fp8", eT)

        def av(c, p2, first, last):
            m = p2 - 4 * c
            qe = 0 if m < 0 else 128 * m
            if first:
                pat_tiles[c] = ps_at.tile(
                    [D + 1, 512], F32, tag="at", name=f"at{c}"
                )
            pat = pat_tiles[c]
            kind, tl = exp_res.pop((c, p2))
            if kind == "u16":
                ebf = tl[:].bitcast(BF16).rearrange(
                    "p (two x) -> p two x", x=512
                )
                for ji in range(2):
                    nc.tensor.matmul(
                        pat[:, 0:512],
                        vobf[:, p2, ji, :],
                        ebf[:, ji, :],
                        start=(first and ji == 0),
                        stop=(last and ji == 1),
                    )
            else:
                eT_v = tl[:].rearrange("p (two x) -> p two x", x=512)
                nc.tensor.matmul(
                    pat[:, qe:512],
                    vo[:, p2, :, :],
                    eT_v[:, :, qe:512],
                    start=first,
                    stop=last,
                    perf_mode=DR,
                )
            if last:
                osb = sb_exp.tile([D + 1, 512], F32, tag="osb", name=f"osb{c}")
                nc.vector.tensor_copy(osb[:], pat[:])
                nc.sync.dma_start(outT[:, c * 512 : (c + 1) * 512], osb[:])

        def mk(fn, *args):
            return lambda: fn(*args)

        # ---------------- schedule ----------------
        # prologue: projections run ~2 key chunks ahead of phase C (all
        # DMAs already in flight above)
        phase_b(0)
        phase_b_q(0)
        phase_b(1)
        phase_b_tr(0)
        phase_b(2)
        phase_b(3)
        phase_b_q(1)

        pairs = []
        for c in range(QC):
            dve_set = set(DVE_PAIRS[c])
            order = (
                list(range(4 * c, 4 * c + 4))
                + [p for p in range(0, 4 * c) if p not in dve_set]
                + [p for p in range(0, 4 * c) if p in dve_set]
            )
            for i, p2 in enumerate(order):
                pairs.append((c, p2, i == 0, i == len(order) - 1))

        inject = {
            1: [mk(phase_b_tr, 1)],
            2: [mk(phase_b, 4)],
            3: [mk(phase_b, 5), mk(phase_b_q, 2)],
            6: [mk(phase_b_tr, 2)],
            8: [mk(phase_b, 6)],
            10: [mk(phase_b, 7), mk(phase_b_q, 3)],
            15: [mk(phase_b_tr, 3)],
        }
        LOOKAHEAD = 2
        for g in range(len(pairs) + LOOKAHEAD):
            if g < len(pairs):
                if g in inject:
                    for fn in inject[g]:
                        fn()
                c, p2, first, last = pairs[g]
                sc_exp(c, p2)
            if g >= LOOKAHEAD:
                c, p2, first, last = pairs[g - LOOKAHEAD]
                av(c, p2, first, last)

    nc.compile()
    return nc


def _stage_inputs(x, Wq, bq, Wk, bk, Wv, bv):
    """Build the 8 per-core input dicts."""
    import ml_dtypes

    NP8 = ml_dtypes.float8_e4m3

    x = np.asarray(x, dtype=np.float32)
    # cols 0:64 = 16*Wk, 64:128 = 16*Wv, 128:192 = 16*Wq
    w3 = np.concatenate(
        [np.asarray(Wk), np.asarray(Wv), np.asarray(Wq)], axis=1
    ).astype(np.float32) * WSCALE

    def dr_fold(mat):
        # [512, C] -> [128, 2, 2, C]: partition = half*64+p, dims (i, t)
        C = mat.shape[1]
        return np.ascontiguousarray(
            mat.reshape(2, 2, 2, 64, C).transpose(0, 3, 1, 2, 4)
            .reshape(P, 2, 2, C)
        )

    w3dr = dr_fold(w3).astype(NP8)

    bias2 = np.zeros((P, 2), dtype=np.float32)
    bias2[0:D, 0] = np.asarray(bk, dtype=np.float32) * WSCALE
    bias2[0:D, 1] = np.asarray(bq, dtype=np.float32) * WSCALE

    in_maps = []
    for core in range(N_CORES):
        b, h = divmod(core, 2)
        g = np.arange(NT)
        if h == 1:
            g = g ^ 1  # pair-swap so query tiles land on even slots
        xb = x[b].reshape(NT, P, E)[g]          # [32,128,512]
        xT = xb.reshape(S, E).T                  # [512, 4096]
        xdr_full = dr_fold(xT).astype(NP8)       # [128, 2, 2, 4096]
        xdr_c = np.ascontiguousarray(
            xdr_full.reshape(P, 2, 2, KC, 512).transpose(3, 0, 1, 2, 4)
        )                                        # [8, 128, 2, 2, 512]
        # queries = even local tiles: chunk c covers local tiles
        # {8c, 8c+2, 8c+4, 8c+6}
        xq_tiles = xdr_full.reshape(P, 2, 2, NT, P)[:, :, :, 0::2, :]
        xq_c = np.ascontiguousarray(
            xq_tiles.reshape(P, 2, 2, QC, 512).transpose(3, 0, 1, 2, 4)
        )                                        # [4, 128, 2, 2, 512]
        # band mask: plane 0 = strict upper tri (key > query in same
        # tile), plane 1 = const (j1 block fully masked for h=0,
        # visible for h=1)
        tri = np.where(
            np.arange(P)[None, :] > np.arange(P)[:, None], MASKVAL, 0.0
        ).astype(np.float32)
        const = np.full((P, P), MASKVAL if h == 0 else 0.0, dtype=np.float32)
        mtri = np.ascontiguousarray(
            np.stack([tri, const], axis=1)
        ).astype(ml_dtypes.bfloat16)
        in_maps.append(
            {
                "xdr": xdr_c,
                "xq": xq_c,
                "w3": w3dr,
                "bias2": bias2,
                "mtri": mtri,
            }
        )
    return in_maps


def _gather_output(results, bv):
    """Merge 8 per-core outT [65, 2048] into the full [B, S, D] output."""
    out = np.empty((B, S, D), dtype=np.float32)
    bv = np.asarray(bv, dtype=np.float32)
    tg = np.array([8 * c + 2 * si for c in range(QC) for si in range(4)])
    for core in range(N_CORES):
        b, h = divmod(core, 2)
        ot = results[core]["outT"]  # [65, 2048]
        attn = ot[0:D] / ot[D : D + 1] / WSCALE + bv[:, None]
        blocks = attn.T.reshape(16, P, D)  # [(c,si), r, d]
        out.reshape(B, NT, P, D)[b, tg + h] = blocks
    return out


def kernel(x, Wq, bq, Wk, bk, Wv, bv):
    if "nc" not in _CACHE:
        _CACHE["nc"] = _build()
    nc = _CACHE["nc"]
    in_maps = _stage_inputs(x, Wq, bq, Wk, bk, Wv, bv)
    res = run_bass_kernel_spmd(nc, in_maps, core_ids=list(range(N_CORES)))
    return _gather_output(res.results, bv)
